# revision 19
# baseline (speedup 1.0000x reference)
"""Multi-head attention (B=2, S=2048, H=1024, NH=16 heads of 64) on 8 trn2
NeuronCores, tensor-parallel over heads with batch parallelism on top.

Sharding: core c handles batch b=c//4 and head-group g=c%4 (4 heads, 256 of
the 1024 hidden cols). Each core computes the partial output
ctx_g @ Wo[g_rows, :]; the host sums the 4 partials per batch and adds the
closed-form bias terms (bv @ Wo + bo; bq/bk are applied on-device).

Device math (per core), fp16 matmul operands, fp32 PSUM accumulation:

  qT/kT = Wq_g^T x_b^T (+bias/partition)  [2 head-pair tiles of 128 x 2048]
  v     = (x_b Wv_g) * exp(mask[k])       [16 tiles of 128 x (4*65)]; col 64
                                          of each head block = exp(mask), so
                                          the ctx matmul also accumulates the
                                          masked softmax denominators and the
                                          attention mask never touches the
                                          exp path (enabling 2048-wide exps)
  scoresT[k,q] = kT.T qT                  (PE; two 1024-col score tiles per
                                           exp: [h0 512q | h1 512q] each)
  expT = exp(0.125*scoresT)               (ACT, 2048 wide: phase 1 pairs the
                                           two live combos at equal kc; phase
                                           2 pairs kc,kc+1 of one combo)
  ctx[q,c] += expT.T v_aug                (PE, stationary=expT slice
                                           [128k x 128q], moving=v [128k x 65]
                                           -> 65-cycle accumulation steps into
                                           a [128q, 260] PSUM tile per (combo
                                           head, 4 qtiles); K-depth is free on
                                           the PE, so this halves ctx cost vs
                                           the moving-exp form AND lands ctx
                                           in [q, d] layout)
  normalize: strided reciprocal of column 64, per-partition mul -> asm[q,:]
  out[q,:] = ctx_n @ Wo_g                 (PE, via transpose to [c,q])

Schedule: phase 1 runs combos (0,0) and (1,0) simultaneously (same kc ->
shared wide exp) while projecting k/v just-in-time; both combos' ctx matmuls
are deferred (exp tiles buffered in SBUF) and flushed as PE filler at the
start of phase 2, where ACT is the steady-state bottleneck. Output tails and
the remaining q projections fill the rest of the phase-2 PE slack.
"""

import os
import sys

sys.path.insert(0, "/opt/trn_rl_repo")

import numpy as np

B, S, H, NH, HD = 2, 2048, 1024, 16, 64
NCORES = 8
HPC = 4          # heads per core
COLS = HPC * HD  # 256
KC = S // 128    # 16 k chunks
QB = 1024        # q block width per combo
NQT = S // 128   # 16 global q tiles
SC = 512         # seq chunk for projections

_CACHE = {}


def _build():
    import concourse.mybir as mybir
    import concourse.tile as tile
    from concourse import bacc
    from concourse.masks import make_identity

    f32 = mybir.dt.float32
    f16 = mybir.dt.float16
    Exp = mybir.ActivationFunctionType.Exp

    nc = bacc.Bacc("TRN2", target_bir_lowering=False, debug=False,
                   num_devices=NCORES)

    xT_d = nc.dram_tensor("xT", [H, S], f16, kind="ExternalInput").ap()
    wq_d = nc.dram_tensor("wq", [H, COLS], f16, kind="ExternalInput").ap()
    wk_d = nc.dram_tensor("wk", [H, COLS], f16, kind="ExternalInput").ap()
    wv_d = nc.dram_tensor("wv", [H, COLS], f16, kind="ExternalInput").ap()
    wo_d = nc.dram_tensor("wo", [COLS, H], f16, kind="ExternalInput").ap()
    bq_d = nc.dram_tensor("bq", [COLS], f32, kind="ExternalInput").ap()
    bk_d = nc.dram_tensor("bk", [COLS], f32, kind="ExternalInput").ap()
    mask_d = nc.dram_tensor("mask", [S], f32, kind="ExternalInput").ap()
    out_d = nc.dram_tensor("out", [S, H], f16, kind="ExternalOutput").ap()

    with tile.TileContext(nc) as tc:
        pers = tc.alloc_tile_pool(name="pers", bufs=1)
        psA = tc.alloc_tile_pool(name="psA", bufs=2, space="PSUM")
        psC = tc.alloc_tile_pool(name="psC", bufs=2, space="PSUM")
        psT = tc.alloc_tile_pool(name="psT", bufs=2, space="PSUM")
        work = tc.alloc_tile_pool(name="work", bufs=3)

        qT = [pers.tile([128, S], f16, tag=f"qT{i}", name=f"qT{i}")
              for i in range(2)]
        kT = [pers.tile([128, S], f16, tag=f"kT{i}", name=f"kT{i}")
              for i in range(2)]
        vt = [pers.tile([128, HPC * 65], f16, tag=f"v{i}", name=f"v{i}")
              for i in range(KC)]
        asm = [pers.tile([128, COLS], f16, tag=f"asm{i}", name=f"asm{i}")
               for i in range(NQT)]
        xt4 = [pers.tile([128, 2 * S], f16, tag=f"xt4{i}", name=f"xt4{i}")
               for i in range(4)]
        wq_a = pers.tile([128, 2048], f16, tag="wq", name="wq_a")
        wk_a = pers.tile([128, 2048], f16, tag="wk", name="wk_a")
        wv_a = pers.tile([128, 2048], f16, tag="wv", name="wv_a")
        wo_a = pers.tile([128, 2048], f16, tag="wo", name="wo_a")

        def xT(hc):
            """View of H-chunk hc of x^T: [128, S] slice of a packed tile."""
            return xt4[hc // 2][:, (hc % 2) * S:(hc % 2) * S + S]
        bq_s = pers.tile([128, 2], f32, tag="bq", name="bq_s")
        bk_s = pers.tile([128, 2], f32, tag="bk", name="bk_s")
        mask_s = pers.tile([128, KC], f32, tag="mask", name="mask_s")
        id128 = pers.tile([128, 128], f16, tag="id128", name="id128")

        warm = pers.tile([1, 1], f32, tag="warm", name="warm")
        nc.gpsimd.memset(warm[:], 0.0)
        nc.scalar.activation(warm[:], warm[:], Exp)
        make_identity(nc, id128[:])
        wps = psT.tile([128, 128], f16, tag="cx", name="wps")
        for _ in range(30):
            nc.tensor.transpose(wps[:], id128[:], id128[:])

        # Few large DMAs on one HWDGE queue (dispatch is ~650ns/DMA), split
        # and ordered so the q-projection pipeline starts as early as
        # possible (queue order = arrival order; subtile deps let the first
        # matmuls start before the later chunks land).
        def xt4_pair(t, lo, hi):
            out = xt4[t].rearrange("p (c s) -> p c s", c=2)[:, :, lo:hi]
            in_ = xT_d[t * 256:(t + 1) * 256, lo:hi].rearrange(
                "(c p) s -> p c s", p=128)
            nc.sync.dma_start(out, in_)

        def w_dma(dst, src, c0, c1):
            nc.sync.dma_start(
                dst.rearrange("p (c n) -> p c n", c=8)[:, c0:c1],
                src.rearrange("(c p) n -> p c n", p=128)[:, c0:c1])

        w_dma(wq_a, wq_d, 0, 4)
        xt4_pair(0, 0, SC)
        w_dma(wk_a, wk_d, 0, 4)
        w_dma(wq_a, wq_d, 4, 8)
        xt4_pair(1, 0, SC)
        nc.sync.dma_start(bq_s[:], bq_d.rearrange("(a p) -> p a", p=128))
        nc.sync.dma_start(bk_s[:], bk_d.rearrange("(a p) -> p a", p=128))
        nc.sync.dma_start(mask_s[:], mask_d.rearrange("(a p) -> p a", p=128))
        xt4_pair(2, 0, SC)
        xt4_pair(3, 0, SC)
        w_dma(wk_a, wk_d, 4, 8)
        w_dma(wv_a, wv_d, 0, 8)
        for t in range(4):
            xt4_pair(t, SC, 2 * SC)
        for t in range(4):
            xt4_pair(t, 2 * SC, S)
        nc.sync.dma_start(wo_a.rearrange("p (c n) -> p c n", c=2),
                          wo_d.rearrange("(c p) n -> p c n", p=128))

        qk_open = {}

        def qk_proj_half(w_a, b_s, dst, pi, sc, half):
            """One half (4 H-chunks) of a q/k projection chain. The two
            halves MUST be emitted with no other psT-using item between
            them (the PSUM tile stays open across the pair)."""
            key = (id(dst), pi, sc)
            if half == 0:
                ps = qk_open[key] = psT.tile([128, SC], f32, tag="cx",
                                             name="pps")
            else:
                ps = qk_open.pop(key)
            for hc in range(4 * half, 4 * half + 4):
                nc.tensor.matmul(
                    ps[:], w_a[:, hc * COLS + pi * 128:hc * COLS + pi * 128 + 128],
                    xT(hc)[:, sc * SC:(sc + 1) * SC],
                    start=(hc == 0), stop=(hc == 7))
            if half == 1:
                nc.vector.tensor_scalar_add(dst[pi][:, sc * SC:(sc + 1) * SC],
                                            ps[:], b_s[:, pi:pi + 1])

        def qk_proj(w_a, b_s, dst, pi, sc):
            qk_proj_half(w_a, b_s, dst, pi, sc, 0)
            qk_proj_half(w_a, b_s, dst, pi, sc, 1)

        def v_proj(st):
            ps = psT.tile([128, COLS], f32, tag="cx", name="vps")
            for hc in range(8):
                nc.tensor.matmul(ps[:], xT(hc)[:, st * 128:(st + 1) * 128],
                                 wv_a[:, hc * COLS:(hc + 1) * COLS],
                                 start=(hc == 0), stop=(hc == 7))
            nc.vector.memset(vt[st][:], 1.0)
            nc.vector.tensor_copy(
                vt[st].rearrange("p (h c) -> p h c", c=65)[:, :, 0:64],
                ps[:].rearrange("p (h c) -> p h c", c=64))

        # ---- attention machinery ----
        ctx_open = {}   # (hp, qb4) -> [j0, j1] open PSUM accumulators
        pend = []       # pending ctx matmuls: (hp, qb4, kc, ex, base)

        def sc_exp(hp, qb4, kc):
            """Scores (two [64,128]x[64,512] matmuls into a 2-bank PSUM tile)
            + one 1024-wide exp; the ctx consumption is deferred via pend."""
            sc_ps = psA.tile([128, QB], f32, tag="sc", name="sc_ps")
            qs = qb4 * 512
            for j in range(2):
                nc.tensor.matmul(
                    sc_ps[:, j * 512:(j + 1) * 512],
                    kT[hp][j * 64:j * 64 + 64, kc * 128:(kc + 1) * 128],
                    qT[hp][j * 64:j * 64 + 64, qs:qs + 512],
                    start=True, stop=True)
            ex = work.tile([128, QB], f16, tag="exp", name="exp", bufs=53)
            nc.scalar.activation(ex[:], sc_ps[:], Exp,
                                 bias=mask_s[:, kc:kc + 1], scale=0.125)
            pend.append((hp, qb4, kc, ex, 0))

        def emit_ctx(hp, qb4, kc, ex, base):
            key = (hp, qb4)
            if key not in ctx_open:
                # lazy: allocated at first flush so the slot-reuse wait lands
                # after the previous combo's normalize has been emitted
                ctx_open[key] = [
                    psC.tile([128, 260], f32, tag="ctx",
                             name=f"ctx{hp}_{qb4}_{j}") for j in range(2)]
            ctxp = ctx_open[key]
            for j in range(2):
                h = hp * 2 + j
                for qt in range(4):
                    nc.tensor.matmul(
                        ctxp[j][:, qt * 65:(qt + 1) * 65],
                        ex[:, base + j * 512 + qt * 128:
                           base + j * 512 + (qt + 1) * 128],
                        vt[kc][:, h * 65:(h + 1) * 65],
                        start=(kc == 0 and qt == 0),
                        stop=(kc == KC - 1 and qt == 3))

        def flush_ctx(hp, qb4, lo, hi):
            """Emit this combo's pending ctx matmuls for kc in [lo, hi) --
            small chunks keep the PE from starving ACT of score tiles."""
            for it in [p for p in pend
                       if (p[0], p[1]) == (hp, qb4) and lo <= p[2] < hi]:
                pend.remove(it)
                emit_ctx(*it)

        def norm_ctx(hp, qb4, tails=False):
            """Normalize the [128q, 65]-tiled PSUM accumulators straight into
            asm (per-partition reciprocal-mul; no transposes). tails=True
            (drain): interleave the output tails qt-major behind the norm
            muls."""
            assert not any((p[0], p[1]) == (hp, qb4) for p in pend)
            ctxp = ctx_open.pop((hp, qb4))
            rcs = []
            for j in range(2):
                rc4 = work.tile([128, 4], f32, tag="rc", name="rc")
                nc.vector.reciprocal(
                    rc4[:],
                    ctxp[j].rearrange("p (q c) -> p q c", c=65)[:, :, 64])
                rcs.append(rc4)
            for qt in range(4):
                for j in range(2):
                    h = hp * 2 + j
                    nc.vector.tensor_scalar_mul(
                        asm[qb4 * 4 + qt][:, h * 64:(h + 1) * 64],
                        ctxp[j][:, qt * 65:qt * 65 + 64],
                        rcs[j][:, qt:qt + 1])
                if tails:
                    tail(qb4, qts=[qt], act=True)

        def tail(qb4, qts=range(4), act=False):
            # act=True (final tail, ACT idle): ctn on ACT, ob on DVE so the
            # per-qt chains pipeline across three engines.
            cp_ctn = nc.vector.tensor_copy
            for qt in qts:
                gqt = qb4 * 4 + qt
                t2p = psT.tile([128, 256], f16, tag="cx", name="t2p")
                for cc in range(2):
                    nc.tensor.transpose(
                        t2p[:, cc * 128:(cc + 1) * 128],
                        asm[gqt][:, cc * 128:(cc + 1) * 128], id128[:])
                ctn = work.tile([128, 256], f16, tag="ctn", name="ctn", bufs=4)
                cp_ctn(ctn[:], t2p[:])
                ob = work.tile([128, H], f16, tag="ob", name="ob", bufs=4)
                ops = []
                for fj in range(2):
                    op = psT.tile([128, 512], f32, tag="cx", name="op")
                    for cc in range(2):
                        nc.tensor.matmul(
                            op[:], ctn[:, cc * 128:(cc + 1) * 128],
                            wo_a[:, cc * H + fj * 512:cc * H + (fj + 1) * 512],
                            start=(cc == 0), stop=(cc == 1))
                    if act:
                        ops.append(op)
                    else:
                        nc.vector.tensor_copy(
                            ob[:, fj * 512:(fj + 1) * 512], op[:])
                if act:
                    # drain: one whole-width ACT copy keeps the per-qt cycle
                    # off the DVE (which runs muls+ctn) and off the PE
                    for fj, op in enumerate(ops):
                        nc.scalar.copy(ob[:, fj * 512:(fj + 1) * 512], op[:])
                nc.sync.dma_start(out_d[gqt * 128:(gqt + 1) * 128, :], ob[:])

        def proj_q(sc):
            for pi in range(2):
                qk_proj(wq_a, bq_s, qT, pi, sc)

        # ---- schedule ----
        # Phase 1: k/v projections just-in-time; three score/exp streams per
        # kc step: (0,0) with live ctx flushing at lag 3 (psC is free here),
        # (1,0) and (0,1) deferred (exp tiles buffered in SBUF). scores(hp,*)
        # only reads the pi=hp half of a q chunk, so (0,1) starts at step 1
        # right after the (pi0, sc1) projection.
        qk_proj(wq_a, bq_s, qT, 0, 0)
        qk_proj(wk_a, bk_s, kT, 0, 0)
        for t in range(KC):
            if t % 4 == 0 and t > 0:
                for pi in range(2):
                    qk_proj(wk_a, bk_s, kT, pi, t // 4)
            sc_exp(0, 0, t)
            if t == 0:
                qk_proj(wq_a, bq_s, qT, 1, 0)
                qk_proj(wk_a, bk_s, kT, 1, 0)
            sc_exp(1, 0, t)
            v_proj(t)
            if t >= 1:
                sc_exp(0, 1, t - 1)
            if t >= 3:
                flush_ctx(0, 0, t - 3, t - 2)
            if t == 0:
                qk_proj(wq_a, bq_s, qT, 0, 1)
        sc_exp(0, 1, KC - 1)
        qk_proj(wq_a, bq_s, qT, 1, 1)
        flush_ctx(0, 0, KC - 3, KC)
        norm_ctx(0, 0)

        # Phase 2: one combo at a time, ACT-bound at one 1024-wide exp per
        # kc. Each combo's ctx flushes during the NEXT combo in 4-kc chunks
        # (one item per kc keeps every PE burst under the exp period);
        # output tails and the deferred q projections fill the rest.
        combos = [(1, 1), (0, 2), (1, 2), (0, 3), (1, 3)]

        def fseq(hp, qb4):
            return [("fc", (hp, qb4, 0, 3)), ("fc", (hp, qb4, 3, 6)),
                    ("fc", (hp, qb4, 6, 9)), ("fc", (hp, qb4, 9, 12)),
                    ("fc", (hp, qb4, 12, 16)), ("n", (hp, qb4))]

        def qh(pi, sc):
            return [("qh", (pi, sc, 0)), ("qh", (pi, sc, 1))]

        fillers = [
            fseq(1, 0) + fseq(0, 1) + [("t", (0, 0))] + qh(0, 2),
            fseq(1, 1) + [("t", (0, 1)), ("t", (0, 2)), ("t", (0, 3))]
            + qh(1, 2),
            fseq(0, 2) + [("t", (1, 0)), ("t", (1, 1))] + qh(0, 3),
            fseq(1, 2) + [("t", (1, 2)), ("t", (1, 3))] + qh(1, 3),
            fseq(0, 3) + [("fc", (1, 3, 0, 3)), ("t", (2, 0)),
                          ("fc", (1, 3, 3, 6)), ("t", (2, 1)),
                          ("fc", (1, 3, 6, 9)), ("t", (2, 2)),
                          ("fc", (1, 3, 9, 12)), ("t", (2, 3)),
                          ("fc", (1, 3, 12, 14))],
        ]

        def emit_item(it):
            kind, arg = it
            if kind == "t":
                tail(arg[0], qts=[arg[1]])
            elif kind == "fc":
                flush_ctx(*arg)
            elif kind == "n":
                norm_ctx(arg[0], arg[1])
            else:
                qk_proj_half(wq_a, bq_s, qT, arg[0], arg[1], arg[2])

        for ci, (hp, qb4) in enumerate(combos):
            items = list(fillers[ci])
            for kc in range(KC):
                sc_exp(hp, qb4, kc)
                if kc >= 1 and items:
                    emit_item(items.pop(0))
            while items:
                emit_item(items.pop(0))
        flush_ctx(1, 3, 14, KC)
        norm_ctx(1, 3, tails=True)

        work.release()
        psT.release()
        psC.release()
        psA.release()
        pers.release()

    nc.compile()
    return nc


def _get_nc():
    if "nc" not in _CACHE:
        _CACHE["nc"] = _build()
    return _CACHE["nc"]


def kernel(hidden_states, attention_mask, Wq, bq, Wk, bk, Wv, bv, Wo, bo):
    from concourse.bass_utils import run_bass_kernel_spmd

    hidden_states = np.asarray(hidden_states, np.float32)
    attention_mask = np.asarray(attention_mask, np.float32)
    Wq, Wk, Wv, Wo = (np.asarray(a, np.float32) for a in (Wq, Wk, Wv, Wo))
    bq, bk, bv, bo = (np.asarray(a, np.float32) for a in (bq, bk, bv, bo))

    nc = _get_nc()
    in_maps = []
    xTb = [np.ascontiguousarray(hidden_states[b].T).astype(np.float16)
           for b in range(B)]
    maskb = [np.ascontiguousarray(attention_mask[b, 0, 0, :])
             for b in range(B)]
    for c in range(NCORES):
        b, g = c // HPC, c % HPC
        cs = slice(g * COLS, (g + 1) * COLS)
        in_maps.append({
            "xT": xTb[b],
            "wq": np.ascontiguousarray(Wq[:, cs]).astype(np.float16),
            "wk": np.ascontiguousarray(Wk[:, cs]).astype(np.float16),
            "wv": np.ascontiguousarray(Wv[:, cs]).astype(np.float16),
            "wo": np.ascontiguousarray(Wo[cs, :]).astype(np.float16),
            "bq": np.ascontiguousarray(bq[cs]),
            "bk": np.ascontiguousarray(bk[cs]),
            "mask": maskb[b],
        })

    trace = bool(os.environ.get("KERNEL_TRACE"))
    kw = {}
    if trace:
        kw = dict(trace=True, tmpdir=os.environ.get("KERNEL_TRACE_DIR"))
    res = run_bass_kernel_spmd(nc, in_maps, list(range(NCORES)), **kw)
    _CACHE["last_result"] = res

    out = np.zeros((B, S, H), np.float32)
    for c in range(NCORES):
        out[c // HPC] += res.results[c]["out"]
    out += bv @ Wo + bo
    return out


# revision 20
# speedup vs baseline: 1.0038x; 1.0038x over previous
"""Multi-head attention (B=2, S=2048, H=1024, NH=16 heads of 64) on 8 trn2
NeuronCores, tensor-parallel over heads with batch parallelism on top.

Sharding: core c handles batch b=c//4 and head-group g=c%4 (4 heads, 256 of
the 1024 hidden cols). Each core computes the partial output
ctx_g @ Wo[g_rows, :]; the host sums the 4 partials per batch and adds the
closed-form bias terms (bv @ Wo + bo; bq/bk are applied on-device).

Device math (per core), fp16 matmul operands, fp32 PSUM accumulation:

  qT/kT = Wq_g^T x_b^T (+bias/partition)  [2 head-pair tiles of 128 x 2048]
  v     = (x_b Wv_g) * exp(mask[k])       [16 tiles of 128 x (4*65)]; col 64
                                          of each head block = exp(mask), so
                                          the ctx matmul also accumulates the
                                          masked softmax denominators and the
                                          attention mask never touches the
                                          exp path (enabling 2048-wide exps)
  scoresT[k,q] = kT.T qT                  (PE; two 1024-col score tiles per
                                           exp: [h0 512q | h1 512q] each)
  expT = exp(0.125*scoresT)               (ACT, 2048 wide: phase 1 pairs the
                                           two live combos at equal kc; phase
                                           2 pairs kc,kc+1 of one combo)
  ctx[q,c] += expT.T v_aug                (PE, stationary=expT slice
                                           [128k x 128q], moving=v [128k x 65]
                                           -> 65-cycle accumulation steps into
                                           a [128q, 260] PSUM tile per (combo
                                           head, 4 qtiles); K-depth is free on
                                           the PE, so this halves ctx cost vs
                                           the moving-exp form AND lands ctx
                                           in [q, d] layout)
  normalize: strided reciprocal of column 64, per-partition mul -> asm[q,:]
  out[q,:] = ctx_n @ Wo_g                 (PE, via transpose to [c,q])

Schedule: phase 1 runs combos (0,0) and (1,0) simultaneously (same kc ->
shared wide exp) while projecting k/v just-in-time; both combos' ctx matmuls
are deferred (exp tiles buffered in SBUF) and flushed as PE filler at the
start of phase 2, where ACT is the steady-state bottleneck. Output tails and
the remaining q projections fill the rest of the phase-2 PE slack.
"""

import os
import sys

sys.path.insert(0, "/opt/trn_rl_repo")

import numpy as np

B, S, H, NH, HD = 2, 2048, 1024, 16, 64
NCORES = 8
HPC = 4          # heads per core
COLS = HPC * HD  # 256
KC = S // 128    # 16 k chunks
QB = 1024        # q block width per combo
NQT = S // 128   # 16 global q tiles
SC = 512         # seq chunk for projections

_CACHE = {}


def _build():
    import concourse.mybir as mybir
    import concourse.tile as tile
    from concourse import bacc
    from concourse.masks import make_identity

    f32 = mybir.dt.float32
    f16 = mybir.dt.float16
    Exp = mybir.ActivationFunctionType.Exp

    nc = bacc.Bacc("TRN2", target_bir_lowering=False, debug=False,
                   num_devices=NCORES)

    xT_d = nc.dram_tensor("xT", [H, S], f16, kind="ExternalInput").ap()
    wq_d = nc.dram_tensor("wq", [H, COLS], f16, kind="ExternalInput").ap()
    wk_d = nc.dram_tensor("wk", [H, COLS], f16, kind="ExternalInput").ap()
    wv_d = nc.dram_tensor("wv", [H, COLS], f16, kind="ExternalInput").ap()
    wo_d = nc.dram_tensor("wo", [COLS, H], f16, kind="ExternalInput").ap()
    bq_d = nc.dram_tensor("bq", [COLS], f32, kind="ExternalInput").ap()
    bk_d = nc.dram_tensor("bk", [COLS], f32, kind="ExternalInput").ap()
    mask_d = nc.dram_tensor("mask", [S], f32, kind="ExternalInput").ap()
    out_d = nc.dram_tensor("out", [S, H], f16, kind="ExternalOutput").ap()

    with tile.TileContext(nc) as tc:
        pers = tc.alloc_tile_pool(name="pers", bufs=1)
        psA = tc.alloc_tile_pool(name="psA", bufs=2, space="PSUM")
        psC = tc.alloc_tile_pool(name="psC", bufs=2, space="PSUM")
        psT = tc.alloc_tile_pool(name="psT", bufs=2, space="PSUM")
        work = tc.alloc_tile_pool(name="work", bufs=3)

        qT = [pers.tile([128, S], f16, tag=f"qT{i}", name=f"qT{i}")
              for i in range(2)]
        kT = [pers.tile([128, S], f16, tag=f"kT{i}", name=f"kT{i}")
              for i in range(2)]
        vt = [pers.tile([128, HPC * 65], f16, tag=f"v{i}", name=f"v{i}")
              for i in range(KC)]
        asm = [pers.tile([128, COLS], f16, tag=f"asm{i}", name=f"asm{i}")
               for i in range(NQT)]
        xt4 = [pers.tile([128, 2 * S], f16, tag=f"xt4{i}", name=f"xt4{i}")
               for i in range(4)]
        wq_a = pers.tile([128, 2048], f16, tag="wq", name="wq_a")
        wk_a = pers.tile([128, 2048], f16, tag="wk", name="wk_a")
        wv_a = pers.tile([128, 2048], f16, tag="wv", name="wv_a")
        wo_a = pers.tile([128, 2048], f16, tag="wo", name="wo_a")

        def xT(hc):
            """View of H-chunk hc of x^T: [128, S] slice of a packed tile."""
            return xt4[hc // 2][:, (hc % 2) * S:(hc % 2) * S + S]
        bq_s = pers.tile([128, 2], f32, tag="bq", name="bq_s")
        bk_s = pers.tile([128, 2], f32, tag="bk", name="bk_s")
        mask_s = pers.tile([128, KC], f32, tag="mask", name="mask_s")
        id128 = pers.tile([128, 128], f16, tag="id128", name="id128")

        warm = pers.tile([1, 1], f32, tag="warm", name="warm")
        nc.gpsimd.memset(warm[:], 0.0)
        nc.scalar.activation(warm[:], warm[:], Exp)
        make_identity(nc, id128[:])
        wps = psT.tile([128, 128], f16, tag="cx", name="wps")
        for _ in range(30):
            nc.tensor.transpose(wps[:], id128[:], id128[:])

        # Few large DMAs on one HWDGE queue (dispatch is ~650ns/DMA), split
        # and ordered so the q-projection pipeline starts as early as
        # possible (queue order = arrival order; subtile deps let the first
        # matmuls start before the later chunks land).
        def xt4_pair(t, lo, hi):
            out = xt4[t].rearrange("p (c s) -> p c s", c=2)[:, :, lo:hi]
            in_ = xT_d[t * 256:(t + 1) * 256, lo:hi].rearrange(
                "(c p) s -> p c s", p=128)
            nc.sync.dma_start(out, in_)

        def w_dma(dst, src, c0, c1):
            nc.sync.dma_start(
                dst.rearrange("p (c n) -> p c n", c=8)[:, c0:c1],
                src.rearrange("(c p) n -> p c n", p=128)[:, c0:c1])

        w_dma(wq_a, wq_d, 0, 4)
        xt4_pair(0, 0, SC)
        w_dma(wk_a, wk_d, 0, 4)
        w_dma(wq_a, wq_d, 4, 8)
        xt4_pair(1, 0, SC)
        nc.sync.dma_start(bq_s[:], bq_d.rearrange("(a p) -> p a", p=128))
        nc.sync.dma_start(bk_s[:], bk_d.rearrange("(a p) -> p a", p=128))
        nc.sync.dma_start(mask_s[:], mask_d.rearrange("(a p) -> p a", p=128))
        xt4_pair(2, 0, SC)
        xt4_pair(3, 0, SC)
        w_dma(wk_a, wk_d, 4, 8)
        w_dma(wv_a, wv_d, 0, 8)
        for t in range(4):
            xt4_pair(t, SC, 2 * SC)
        for t in range(4):
            xt4_pair(t, 2 * SC, S)
        nc.sync.dma_start(wo_a.rearrange("p (c n) -> p c n", c=2),
                          wo_d.rearrange("(c p) n -> p c n", p=128))

        qk_open = {}

        def qk_proj_half(w_a, b_s, dst, pi, sc, half):
            """One half (4 H-chunks) of a q/k projection chain. The two
            halves MUST be emitted with no other psT-using item between
            them (the PSUM tile stays open across the pair)."""
            key = (id(dst), pi, sc)
            if half == 0:
                ps = qk_open[key] = psT.tile([128, SC], f32, tag="cx",
                                             name="pps")
            else:
                ps = qk_open.pop(key)
            for hc in range(4 * half, 4 * half + 4):
                nc.tensor.matmul(
                    ps[:], w_a[:, hc * COLS + pi * 128:hc * COLS + pi * 128 + 128],
                    xT(hc)[:, sc * SC:(sc + 1) * SC],
                    start=(hc == 0), stop=(hc == 7))
            if half == 1:
                nc.vector.tensor_scalar_add(dst[pi][:, sc * SC:(sc + 1) * SC],
                                            ps[:], b_s[:, pi:pi + 1])

        def qk_proj(w_a, b_s, dst, pi, sc):
            qk_proj_half(w_a, b_s, dst, pi, sc, 0)
            qk_proj_half(w_a, b_s, dst, pi, sc, 1)

        def v_proj(st):
            ps = psT.tile([128, COLS], f32, tag="cx", name="vps")
            for hc in range(8):
                nc.tensor.matmul(ps[:], xT(hc)[:, st * 128:(st + 1) * 128],
                                 wv_a[:, hc * COLS:(hc + 1) * COLS],
                                 start=(hc == 0), stop=(hc == 7))
            nc.vector.memset(vt[st][:], 1.0)
            nc.vector.tensor_copy(
                vt[st].rearrange("p (h c) -> p h c", c=65)[:, :, 0:64],
                ps[:].rearrange("p (h c) -> p h c", c=64))

        # ---- attention machinery ----
        ctx_open = {}   # (hp, qb4) -> [j0, j1] open PSUM accumulators
        pend = []       # pending ctx matmuls: (hp, qb4, kc, ex, base)

        def sc_exp(hp, qb4, kc):
            """Scores (two [64,128]x[64,512] matmuls into a 2-bank PSUM tile)
            + one 1024-wide exp; the ctx consumption is deferred via pend."""
            sc_ps = psA.tile([128, QB], f32, tag="sc", name="sc_ps")
            qs = qb4 * 512
            for j in range(2):
                nc.tensor.matmul(
                    sc_ps[:, j * 512:(j + 1) * 512],
                    kT[hp][j * 64:j * 64 + 64, kc * 128:(kc + 1) * 128],
                    qT[hp][j * 64:j * 64 + 64, qs:qs + 512],
                    start=True, stop=True)
            ex = work.tile([128, QB], f16, tag="exp", name="exp", bufs=53)
            nc.scalar.activation(ex[:], sc_ps[:], Exp,
                                 bias=mask_s[:, kc:kc + 1], scale=0.125)
            pend.append((hp, qb4, kc, ex, 0))

        def emit_ctx(hp, qb4, kc, ex, base):
            key = (hp, qb4)
            if key not in ctx_open:
                # lazy: allocated at first flush so the slot-reuse wait lands
                # after the previous combo's normalize has been emitted
                ctx_open[key] = [
                    psC.tile([128, 260], f32, tag="ctx",
                             name=f"ctx{hp}_{qb4}_{j}") for j in range(2)]
            ctxp = ctx_open[key]
            for j in range(2):
                h = hp * 2 + j
                for qt in range(4):
                    nc.tensor.matmul(
                        ctxp[j][:, qt * 65:(qt + 1) * 65],
                        ex[:, base + j * 512 + qt * 128:
                           base + j * 512 + (qt + 1) * 128],
                        vt[kc][:, h * 65:(h + 1) * 65],
                        start=(kc == 0 and qt == 0),
                        stop=(kc == KC - 1 and qt == 3))

        def flush_ctx(hp, qb4, lo, hi):
            """Emit this combo's pending ctx matmuls for kc in [lo, hi) --
            small chunks keep the PE from starving ACT of score tiles."""
            for it in [p for p in pend
                       if (p[0], p[1]) == (hp, qb4) and lo <= p[2] < hi]:
                pend.remove(it)
                emit_ctx(*it)

        def norm_ctx(hp, qb4, tails=False):
            """Normalize the [128q, 65]-tiled PSUM accumulators straight into
            asm (per-partition reciprocal-mul; no transposes). tails=True
            (drain): interleave the output tails qt-major behind the norm
            muls."""
            assert not any((p[0], p[1]) == (hp, qb4) for p in pend)
            ctxp = ctx_open.pop((hp, qb4))
            rcs = []
            for j in range(2):
                rc4 = work.tile([128, 4], f32, tag="rc", name="rc")
                nc.vector.reciprocal(
                    rc4[:],
                    ctxp[j].rearrange("p (q c) -> p q c", c=65)[:, :, 64])
                rcs.append(rc4)
            for qt in range(4):
                for j in range(2):
                    h = hp * 2 + j
                    nc.vector.tensor_scalar_mul(
                        asm[qb4 * 4 + qt][:, h * 64:(h + 1) * 64],
                        ctxp[j][:, qt * 65:qt * 65 + 64],
                        rcs[j][:, qt:qt + 1])
            if tails:
                tail(qb4, act=True)

        def tail(qb4, qts=range(4), act=False):
            # act=True (final tail, ACT idle): ctn on ACT, ob on DVE so the
            # per-qt chains pipeline across three engines.
            for qt in qts:
                gqt = qb4 * 4 + qt
                if act:
                    # drain: scores/ctx PSUM pools are free; using them (and
                    # a single 2-bank op tile) removes all slot-reuse stalls
                    t2p = psC.tile([128, 256], f16, tag="ctx", name="t2p")
                else:
                    t2p = psT.tile([128, 256], f16, tag="cx", name="t2p")
                for cc in range(2):
                    nc.tensor.transpose(
                        t2p[:, cc * 128:(cc + 1) * 128],
                        asm[gqt][:, cc * 128:(cc + 1) * 128], id128[:])
                ctn = work.tile([128, 256], f16, tag="ctn", name="ctn", bufs=4)
                nc.vector.tensor_copy(ctn[:], t2p[:])
                ob = work.tile([128, H], f16, tag="ob", name="ob", bufs=4)
                if act:
                    op = psA.tile([128, QB], f32, tag="sc", name="opw")
                    for fj in range(2):
                        for cc in range(2):
                            nc.tensor.matmul(
                                op[:, fj * 512:(fj + 1) * 512],
                                ctn[:, cc * 128:(cc + 1) * 128],
                                wo_a[:, cc * H + fj * 512:cc * H + (fj + 1) * 512],
                                start=(cc == 0), stop=(cc == 1))
                    nc.scalar.copy(ob[:, 0:512], op[:, 0:512])
                    nc.vector.tensor_copy(ob[:, 512:H], op[:, 512:H])
                else:
                    for fj in range(2):
                        op = psT.tile([128, 512], f32, tag="cx", name="op")
                        for cc in range(2):
                            nc.tensor.matmul(
                                op[:], ctn[:, cc * 128:(cc + 1) * 128],
                                wo_a[:, cc * H + fj * 512:cc * H + (fj + 1) * 512],
                                start=(cc == 0), stop=(cc == 1))
                        nc.vector.tensor_copy(
                            ob[:, fj * 512:(fj + 1) * 512], op[:])
                nc.sync.dma_start(out_d[gqt * 128:(gqt + 1) * 128, :], ob[:])

        def proj_q(sc):
            for pi in range(2):
                qk_proj(wq_a, bq_s, qT, pi, sc)

        # ---- schedule ----
        # Phase 1: k/v projections just-in-time; three score/exp streams per
        # kc step: (0,0) with live ctx flushing at lag 3 (psC is free here),
        # (1,0) and (0,1) deferred (exp tiles buffered in SBUF). scores(hp,*)
        # only reads the pi=hp half of a q chunk, so (0,1) starts at step 1
        # right after the (pi0, sc1) projection.
        qk_proj(wq_a, bq_s, qT, 0, 0)
        qk_proj(wk_a, bk_s, kT, 0, 0)
        for t in range(KC):
            if t % 4 == 0 and t > 0:
                for pi in range(2):
                    qk_proj(wk_a, bk_s, kT, pi, t // 4)
            sc_exp(0, 0, t)
            if t == 0:
                qk_proj(wq_a, bq_s, qT, 1, 0)
                qk_proj(wk_a, bk_s, kT, 1, 0)
            sc_exp(1, 0, t)
            v_proj(t)
            if t >= 1:
                sc_exp(0, 1, t - 1)
            if t >= 3:
                flush_ctx(0, 0, t - 3, t - 2)
            if t == 0:
                qk_proj(wq_a, bq_s, qT, 0, 1)
        sc_exp(0, 1, KC - 1)
        qk_proj(wq_a, bq_s, qT, 1, 1)
        flush_ctx(0, 0, KC - 3, KC)
        norm_ctx(0, 0)

        # Phase 2: one combo at a time, ACT-bound at one 1024-wide exp per
        # kc. Each combo's ctx flushes during the NEXT combo in 4-kc chunks
        # (one item per kc keeps every PE burst under the exp period);
        # output tails and the deferred q projections fill the rest.
        combos = [(1, 1), (0, 2), (1, 2), (0, 3), (1, 3)]

        def fseq(hp, qb4):
            return [("fc", (hp, qb4, 0, 3)), ("fc", (hp, qb4, 3, 6)),
                    ("fc", (hp, qb4, 6, 9)), ("fc", (hp, qb4, 9, 12)),
                    ("fc", (hp, qb4, 12, 16)), ("n", (hp, qb4))]

        def qh(pi, sc):
            return [("qh", (pi, sc, 0)), ("qh", (pi, sc, 1))]

        fillers = [
            fseq(1, 0) + fseq(0, 1) + [("t", (0, 0))] + qh(0, 2),
            fseq(1, 1) + [("t", (0, 1)), ("t", (0, 2)), ("t", (0, 3))]
            + qh(1, 2),
            fseq(0, 2) + [("t", (1, 0)), ("t", (1, 1))] + qh(0, 3),
            fseq(1, 2) + [("t", (1, 2)), ("t", (1, 3))] + qh(1, 3),
            fseq(0, 3) + [("fc", (1, 3, 0, 3)), ("t", (2, 0)),
                          ("fc", (1, 3, 3, 6)), ("t", (2, 1)),
                          ("fc", (1, 3, 6, 9)), ("t", (2, 2)),
                          ("fc", (1, 3, 9, 12)), ("t", (2, 3)),
                          ("fc", (1, 3, 12, 14))],
        ]

        def emit_item(it):
            kind, arg = it
            if kind == "t":
                tail(arg[0], qts=[arg[1]])
            elif kind == "fc":
                flush_ctx(*arg)
            elif kind == "n":
                norm_ctx(arg[0], arg[1])
            else:
                qk_proj_half(wq_a, bq_s, qT, arg[0], arg[1], arg[2])

        for ci, (hp, qb4) in enumerate(combos):
            items = list(fillers[ci])
            for kc in range(KC):
                sc_exp(hp, qb4, kc)
                if kc >= 1 and items:
                    emit_item(items.pop(0))
            while items:
                emit_item(items.pop(0))
        flush_ctx(1, 3, 14, KC)
        norm_ctx(1, 3, tails=True)

        work.release()
        psT.release()
        psC.release()
        psA.release()
        pers.release()

    nc.compile()
    return nc


def _get_nc():
    if "nc" not in _CACHE:
        _CACHE["nc"] = _build()
    return _CACHE["nc"]


def kernel(hidden_states, attention_mask, Wq, bq, Wk, bk, Wv, bv, Wo, bo):
    from concourse.bass_utils import run_bass_kernel_spmd

    hidden_states = np.asarray(hidden_states, np.float32)
    attention_mask = np.asarray(attention_mask, np.float32)
    Wq, Wk, Wv, Wo = (np.asarray(a, np.float32) for a in (Wq, Wk, Wv, Wo))
    bq, bk, bv, bo = (np.asarray(a, np.float32) for a in (bq, bk, bv, bo))

    nc = _get_nc()
    in_maps = []
    xTb = [np.ascontiguousarray(hidden_states[b].T).astype(np.float16)
           for b in range(B)]
    maskb = [np.ascontiguousarray(attention_mask[b, 0, 0, :])
             for b in range(B)]
    for c in range(NCORES):
        b, g = c // HPC, c % HPC
        cs = slice(g * COLS, (g + 1) * COLS)
        in_maps.append({
            "xT": xTb[b],
            "wq": np.ascontiguousarray(Wq[:, cs]).astype(np.float16),
            "wk": np.ascontiguousarray(Wk[:, cs]).astype(np.float16),
            "wv": np.ascontiguousarray(Wv[:, cs]).astype(np.float16),
            "wo": np.ascontiguousarray(Wo[cs, :]).astype(np.float16),
            "bq": np.ascontiguousarray(bq[cs]),
            "bk": np.ascontiguousarray(bk[cs]),
            "mask": maskb[b],
        })

    trace = bool(os.environ.get("KERNEL_TRACE"))
    kw = {}
    if trace:
        kw = dict(trace=True, tmpdir=os.environ.get("KERNEL_TRACE_DIR"))
    res = run_bass_kernel_spmd(nc, in_maps, list(range(NCORES)), **kw)
    _CACHE["last_result"] = res

    out = np.zeros((B, S, H), np.float32)
    for c in range(NCORES):
        out[c // HPC] += res.results[c]["out"]
    out += bv @ Wo + bo
    return out


# revision 22
# speedup vs baseline: 1.0259x; 1.0221x over previous
"""Multi-head attention (B=2, S=2048, H=1024, NH=16 heads of 64) on 8 trn2
NeuronCores, tensor-parallel over heads with batch parallelism on top.

Sharding: core c handles batch b=c//4 and head-group g=c%4 (4 heads, 256 of
the 1024 hidden cols). Each core computes the partial output
ctx_g @ Wo[g_rows, :]; the host sums the 4 partials per batch and adds the
closed-form bias terms (bv @ Wo + bo; bq/bk are applied on-device).

Device math (per core), fp16 matmul operands, fp32 PSUM accumulation:

  qT/kT = Wq_g^T x_b^T (+bias/partition)  [2 head-pair tiles of 128 x 2048]
  v     = (x_b Wv_g) * exp(mask[k])       [16 tiles of 128 x (4*65)]; col 64
                                          of each head block = exp(mask), so
                                          the ctx matmul also accumulates the
                                          masked softmax denominators and the
                                          attention mask never touches the
                                          exp path (enabling 2048-wide exps)
  scoresT[k,q] = kT.T qT                  (PE; two 1024-col score tiles per
                                           exp: [h0 512q | h1 512q] each)
  expT = exp(0.125*scoresT)               (ACT, 2048 wide: phase 1 pairs the
                                           two live combos at equal kc; phase
                                           2 pairs kc,kc+1 of one combo)
  ctx[q,c] += expT.T v_aug                (PE, stationary=expT slice
                                           [128k x 128q], moving=v [128k x 65]
                                           -> 65-cycle accumulation steps into
                                           a [128q, 260] PSUM tile per (combo
                                           head, 4 qtiles); K-depth is free on
                                           the PE, so this halves ctx cost vs
                                           the moving-exp form AND lands ctx
                                           in [q, d] layout)
  normalize: strided reciprocal of column 64, per-partition mul -> asm[q,:]
  out[q,:] = ctx_n @ Wo_g                 (PE, via transpose to [c,q])

Schedule: phase 1 runs combos (0,0) and (1,0) simultaneously (same kc ->
shared wide exp) while projecting k/v just-in-time; both combos' ctx matmuls
are deferred (exp tiles buffered in SBUF) and flushed as PE filler at the
start of phase 2, where ACT is the steady-state bottleneck. Output tails and
the remaining q projections fill the rest of the phase-2 PE slack.
"""

import os
import sys

sys.path.insert(0, "/opt/trn_rl_repo")

import numpy as np

B, S, H, NH, HD = 2, 2048, 1024, 16, 64
NCORES = 8
HPC = 4          # heads per core
COLS = HPC * HD  # 256
KC = S // 128    # 16 k chunks
QB = 1024        # q block width per combo
NQT = S // 128   # 16 global q tiles
SC = 512         # seq chunk for projections

_CACHE = {}


def _build():
    import concourse.mybir as mybir
    import concourse.tile as tile
    from concourse import bacc
    from concourse.masks import make_identity

    f32 = mybir.dt.float32
    f16 = mybir.dt.float16
    Exp = mybir.ActivationFunctionType.Exp

    nc = bacc.Bacc("TRN2", target_bir_lowering=False, debug=False,
                   num_devices=NCORES)

    xT_d = nc.dram_tensor("xT", [H, S], f16, kind="ExternalInput").ap()
    wq_d = nc.dram_tensor("wq", [H, COLS], f16, kind="ExternalInput").ap()
    wk_d = nc.dram_tensor("wk", [H, COLS], f16, kind="ExternalInput").ap()
    wv_d = nc.dram_tensor("wv", [H, COLS], f16, kind="ExternalInput").ap()
    wo_d = nc.dram_tensor("wo", [COLS, H], f16, kind="ExternalInput").ap()
    bq_d = nc.dram_tensor("bq", [COLS], f32, kind="ExternalInput").ap()
    bk_d = nc.dram_tensor("bk", [COLS], f32, kind="ExternalInput").ap()
    mask_d = nc.dram_tensor("mask", [S], f32, kind="ExternalInput").ap()
    out_d = nc.dram_tensor("out", [S, H], f16, kind="ExternalOutput").ap()

    with tile.TileContext(nc) as tc:
        pers = tc.alloc_tile_pool(name="pers", bufs=1)
        psA = tc.alloc_tile_pool(name="psA", bufs=2, space="PSUM")
        psC = tc.alloc_tile_pool(name="psC", bufs=2, space="PSUM")
        psT = tc.alloc_tile_pool(name="psT", bufs=2, space="PSUM")
        work = tc.alloc_tile_pool(name="work", bufs=3)

        qT = [pers.tile([128, S], f16, tag=f"qT{i}", name=f"qT{i}")
              for i in range(2)]
        kT = [pers.tile([128, S], f16, tag=f"kT{i}", name=f"kT{i}")
              for i in range(2)]
        vt = [pers.tile([128, HPC * 65], f16, tag=f"v{i}", name=f"v{i}")
              for i in range(KC)]
        asm = [pers.tile([128, COLS], f16, tag=f"asm{i}", name=f"asm{i}")
               for i in range(NQT)]
        xt4 = [pers.tile([128, 2 * S], f16, tag=f"xt4{i}", name=f"xt4{i}")
               for i in range(4)]
        wq_a = pers.tile([128, 2048], f16, tag="wq", name="wq_a")
        wk_a = pers.tile([128, 2048], f16, tag="wk", name="wk_a")
        wv_a = pers.tile([128, 2048], f16, tag="wv", name="wv_a")
        wo_a = pers.tile([128, 2048], f16, tag="wo", name="wo_a")

        def xT(hc):
            """View of H-chunk hc of x^T: [128, S] slice of a packed tile."""
            return xt4[hc // 2][:, (hc % 2) * S:(hc % 2) * S + S]
        bq_s = pers.tile([128, 2], f32, tag="bq", name="bq_s")
        bk_s = pers.tile([128, 2], f32, tag="bk", name="bk_s")
        mask_s = pers.tile([128, KC], f32, tag="mask", name="mask_s")
        id128 = pers.tile([128, 128], f16, tag="id128", name="id128")

        warm = pers.tile([1, 1], f32, tag="warm", name="warm")
        nc.gpsimd.memset(warm[:], 0.0)
        nc.scalar.activation(warm[:], warm[:], Exp)
        make_identity(nc, id128[:])
        wps = psT.tile([128, 128], f16, tag="cx", name="wps")
        for _ in range(30):
            nc.tensor.transpose(wps[:], id128[:], id128[:])

        # Few large DMAs on one HWDGE queue (dispatch is ~650ns/DMA), split
        # and ordered so the q-projection pipeline starts as early as
        # possible (queue order = arrival order; subtile deps let the first
        # matmuls start before the later chunks land).
        def xt4_pair(t, lo, hi):
            out = xt4[t].rearrange("p (c s) -> p c s", c=2)[:, :, lo:hi]
            in_ = xT_d[t * 256:(t + 1) * 256, lo:hi].rearrange(
                "(c p) s -> p c s", p=128)
            nc.sync.dma_start(out, in_)

        def w_dma(dst, src, c0, c1):
            nc.sync.dma_start(
                dst.rearrange("p (c n) -> p c n", c=8)[:, c0:c1],
                src.rearrange("(c p) n -> p c n", p=128)[:, c0:c1])

        w_dma(wq_a, wq_d, 0, 4)
        xt4_pair(0, 0, SC)
        w_dma(wk_a, wk_d, 0, 4)
        w_dma(wq_a, wq_d, 4, 8)
        xt4_pair(1, 0, SC)
        nc.sync.dma_start(bq_s[:], bq_d.rearrange("(a p) -> p a", p=128))
        nc.sync.dma_start(bk_s[:], bk_d.rearrange("(a p) -> p a", p=128))
        nc.sync.dma_start(mask_s[:], mask_d.rearrange("(a p) -> p a", p=128))
        xt4_pair(2, 0, SC)
        xt4_pair(3, 0, SC)
        w_dma(wk_a, wk_d, 4, 8)
        w_dma(wv_a, wv_d, 0, 8)
        for t in range(4):
            xt4_pair(t, SC, 2 * SC)
        for t in range(4):
            xt4_pair(t, 2 * SC, S)
        nc.sync.dma_start(wo_a.rearrange("p (c n) -> p c n", c=2),
                          wo_d.rearrange("(c p) n -> p c n", p=128))

        qk_open = {}

        def qk_proj_half(w_a, b_s, dst, pi, sc, half):
            """One half (4 H-chunks) of a q/k projection chain. The two
            halves MUST be emitted with no other psT-using item between
            them (the PSUM tile stays open across the pair)."""
            key = (id(dst), pi, sc)
            if half == 0:
                ps = qk_open[key] = psT.tile([128, SC], f32, tag="cx",
                                             name="pps")
            else:
                ps = qk_open.pop(key)
            for hc in range(4 * half, 4 * half + 4):
                nc.tensor.matmul(
                    ps[:], w_a[:, hc * COLS + pi * 128:hc * COLS + pi * 128 + 128],
                    xT(hc)[:, sc * SC:(sc + 1) * SC],
                    start=(hc == 0), stop=(hc == 7))
            if half == 1:
                nc.vector.tensor_scalar_add(dst[pi][:, sc * SC:(sc + 1) * SC],
                                            ps[:], b_s[:, pi:pi + 1])

        def qk_proj(w_a, b_s, dst, pi, sc):
            qk_proj_half(w_a, b_s, dst, pi, sc, 0)
            qk_proj_half(w_a, b_s, dst, pi, sc, 1)

        def v_proj(st):
            ps = psT.tile([128, COLS], f32, tag="cx", name="vps")
            for hc in range(8):
                nc.tensor.matmul(ps[:], xT(hc)[:, st * 128:(st + 1) * 128],
                                 wv_a[:, hc * COLS:(hc + 1) * COLS],
                                 start=(hc == 0), stop=(hc == 7))
            nc.vector.memset(vt[st][:], 1.0)
            nc.vector.tensor_copy(
                vt[st].rearrange("p (h c) -> p h c", c=65)[:, :, 0:64],
                ps[:].rearrange("p (h c) -> p h c", c=64))

        # ---- attention machinery ----
        ctx_open = {}   # (hp, qb4) -> [j0, j1] open PSUM accumulators
        pend = []       # pending ctx matmuls: (hp, qb4, kc, ex, base)

        def sc_exp(hp, qb4, kc):
            """Scores (two [64,128]x[64,512] matmuls into a 2-bank PSUM tile)
            + one 1024-wide exp; the ctx consumption is deferred via pend."""
            sc_ps = psA.tile([128, QB], f32, tag="sc", name="sc_ps")
            qs = qb4 * 512
            for j in range(2):
                nc.tensor.matmul(
                    sc_ps[:, j * 512:(j + 1) * 512],
                    kT[hp][j * 64:j * 64 + 64, kc * 128:(kc + 1) * 128],
                    qT[hp][j * 64:j * 64 + 64, qs:qs + 512],
                    start=True, stop=True)
            ex = work.tile([128, QB], f16, tag="exp", name="exp", bufs=53)
            nc.scalar.activation(ex[:], sc_ps[:], Exp,
                                 bias=mask_s[:, kc:kc + 1], scale=0.125)
            pend.append((hp, qb4, kc, ex, 0))

        def emit_ctx(hp, qb4, kc, ex, base):
            key = (hp, qb4)
            if key not in ctx_open:
                # lazy: allocated at first flush so the slot-reuse wait lands
                # after the previous combo's normalize has been emitted
                ctx_open[key] = [
                    psC.tile([128, 260], f32, tag="ctx",
                             name=f"ctx{hp}_{qb4}_{j}") for j in range(2)]
            ctxp = ctx_open[key]
            for j in range(2):
                h = hp * 2 + j
                for qt in range(4):
                    nc.tensor.matmul(
                        ctxp[j][:, qt * 65:(qt + 1) * 65],
                        ex[:, base + j * 512 + qt * 128:
                           base + j * 512 + (qt + 1) * 128],
                        vt[kc][:, h * 65:(h + 1) * 65],
                        start=(kc == 0 and qt == 0),
                        stop=(kc == KC - 1 and qt == 3))

        def flush_ctx(hp, qb4, lo, hi):
            """Emit this combo's pending ctx matmuls for kc in [lo, hi) --
            small chunks keep the PE from starving ACT of score tiles."""
            for it in [p for p in pend
                       if (p[0], p[1]) == (hp, qb4) and lo <= p[2] < hi]:
                pend.remove(it)
                emit_ctx(*it)

        def norm_ctx(hp, qb4, tails=False):
            """Normalize the [128q, 65]-tiled PSUM accumulators straight into
            asm (per-partition reciprocal-mul; no transposes). tails=True
            (drain): interleave the output tails qt-major behind the norm
            muls."""
            assert not any((p[0], p[1]) == (hp, qb4) for p in pend)
            ctxp = ctx_open.pop((hp, qb4))
            rcs = []
            for j in range(2):
                rc4 = work.tile([128, 4], f32, tag="rc", name="rc")
                nc.vector.reciprocal(
                    rc4[:],
                    ctxp[j].rearrange("p (q c) -> p q c", c=65)[:, :, 64])
                rcs.append(rc4)
            for qt in range(4):
                for j in range(2):
                    h = hp * 2 + j
                    if tails and j == 1:
                        nc.scalar.mul(
                            asm[qb4 * 4 + qt][:, h * 64:(h + 1) * 64],
                            ctxp[j][:, qt * 65:qt * 65 + 64],
                            rcs[j][:, qt:qt + 1])
                    else:
                        nc.vector.tensor_scalar_mul(
                            asm[qb4 * 4 + qt][:, h * 64:(h + 1) * 64],
                            ctxp[j][:, qt * 65:qt * 65 + 64],
                            rcs[j][:, qt:qt + 1])
            if tails:
                tail(qb4, act=True)

        def tail(qb4, qts=range(4), act=False):
            # act=True (final tail, ACT idle): ctn on ACT, ob on DVE so the
            # per-qt chains pipeline across three engines.
            for qt in qts:
                gqt = qb4 * 4 + qt
                if act:
                    # drain: scores/ctx PSUM pools are free; using them (and
                    # a single 2-bank op tile) removes all slot-reuse stalls
                    t2p = psC.tile([128, 256], f16, tag="ctx", name="t2p")
                else:
                    t2p = psT.tile([128, 256], f16, tag="cx", name="t2p")
                for cc in range(2):
                    nc.tensor.transpose(
                        t2p[:, cc * 128:(cc + 1) * 128],
                        asm[gqt][:, cc * 128:(cc + 1) * 128], id128[:])
                ctn = work.tile([128, 256], f16, tag="ctn", name="ctn", bufs=4)
                nc.vector.tensor_copy(ctn[:], t2p[:])
                ob = work.tile([128, H], f16, tag="ob", name="ob", bufs=4)
                if act:
                    op = psA.tile([128, QB], f32, tag="sc", name="opw")
                    for fj in range(2):
                        for cc in range(2):
                            nc.tensor.matmul(
                                op[:, fj * 512:(fj + 1) * 512],
                                ctn[:, cc * 128:(cc + 1) * 128],
                                wo_a[:, cc * H + fj * 512:cc * H + (fj + 1) * 512],
                                start=(cc == 0), stop=(cc == 1))
                    nc.scalar.copy(ob[:, 0:512], op[:, 0:512])
                    nc.vector.tensor_copy(ob[:, 512:H], op[:, 512:H])
                else:
                    for fj in range(2):
                        op = psT.tile([128, 512], f32, tag="cx", name="op")
                        for cc in range(2):
                            nc.tensor.matmul(
                                op[:], ctn[:, cc * 128:(cc + 1) * 128],
                                wo_a[:, cc * H + fj * 512:cc * H + (fj + 1) * 512],
                                start=(cc == 0), stop=(cc == 1))
                        nc.vector.tensor_copy(
                            ob[:, fj * 512:(fj + 1) * 512], op[:])
                nc.sync.dma_start(out_d[gqt * 128:(gqt + 1) * 128, :], ob[:])

        def proj_q(sc):
            for pi in range(2):
                qk_proj(wq_a, bq_s, qT, pi, sc)

        # ---- schedule ----
        # Phase 1: k/v projections just-in-time; three score/exp streams per
        # kc step: (0,0) with live ctx flushing at lag 3 (psC is free here),
        # (1,0) and (0,1) deferred (exp tiles buffered in SBUF). scores(hp,*)
        # only reads the pi=hp half of a q chunk, so (0,1) starts at step 1
        # right after the (pi0, sc1) projection.
        qk_proj(wq_a, bq_s, qT, 0, 0)
        qk_proj(wk_a, bk_s, kT, 0, 0)
        for t in range(KC):
            if t % 4 == 0 and t > 0:
                for pi in range(2):
                    qk_proj(wk_a, bk_s, kT, pi, t // 4)
            sc_exp(0, 0, t)
            if t == 0:
                qk_proj(wq_a, bq_s, qT, 1, 0)
                qk_proj(wk_a, bk_s, kT, 1, 0)
            sc_exp(1, 0, t)
            if t < KC - 2:
                v_proj(t)
            if t >= 1:
                sc_exp(0, 1, t - 1)
            if t >= 3:
                flush_ctx(0, 0, t - 3, t - 2)
            if t == 0:
                qk_proj(wq_a, bq_s, qT, 0, 1)
        sc_exp(0, 1, KC - 1)
        qk_proj(wq_a, bq_s, qT, 1, 1)
        flush_ctx(0, 0, KC - 3, KC - 2)

        # Phase 2: one combo at a time, ACT-bound at one 1024-wide exp per
        # kc. Each combo's ctx flushes during the NEXT combo in 4-kc chunks
        # (one item per kc keeps every PE burst under the exp period);
        # output tails and the deferred q projections fill the rest.
        combos = [(1, 1), (0, 2), (1, 2), (0, 3), (1, 3)]

        def fseq(hp, qb4):
            return [("fc", (hp, qb4, 0, 3)), ("fc", (hp, qb4, 3, 6)),
                    ("fc", (hp, qb4, 6, 9)), ("fc", (hp, qb4, 9, 12)),
                    ("fc", (hp, qb4, 12, 16)), ("n", (hp, qb4))]

        def qh(pi, sc):
            return [("qh", (pi, sc, 0)), ("qh", (pi, sc, 1))]

        fillers = [
            [("v", (14,)), ("v", (15,)), ("fc", (0, 0, 14, 16)),
             ("n", (0, 0)), ("fc", (1, 0, 0, 3)), ("fc", (1, 0, 3, 6)),
             ("fc", (1, 0, 6, 9)), ("fc", (1, 0, 9, 12)),
             ("fc", (1, 0, 12, 16)), ("n", (1, 0)), ("fc", (0, 1, 0, 3)),
             ("fc", (0, 1, 3, 6)), ("fc", (0, 1, 6, 9))] + qh(0, 2),
            [("fc", (0, 1, 9, 12)), ("fc", (0, 1, 12, 16)), ("n", (0, 1)),
             ("fc", (1, 1, 0, 3)), ("fc", (1, 1, 3, 6)), ("fc", (1, 1, 6, 9)),
             ("fc", (1, 1, 9, 12)), ("t", (0, 0)), ("t", (0, 1))]
            + qh(1, 2),
            [("fc", (1, 1, 12, 16)), ("n", (1, 1)), ("fc", (0, 2, 0, 3)),
             ("fc", (0, 2, 3, 6)), ("fc", (0, 2, 6, 9)), ("fc", (0, 2, 9, 12)),
             ("t", (0, 2)), ("t", (0, 3)), ("t", (1, 0))] + qh(0, 3),
            [("fc", (0, 2, 12, 16)), ("n", (0, 2)), ("fc", (1, 2, 0, 3)),
             ("fc", (1, 2, 3, 6)), ("fc", (1, 2, 6, 9)), ("fc", (1, 2, 9, 12)),
             ("t", (1, 1)), ("t", (1, 2)), ("t", (1, 3))] + qh(1, 3),
            [("fc", (1, 2, 12, 16)), ("n", (1, 2)), ("fc", (0, 3, 0, 3)),
             ("fc", (0, 3, 3, 6)), ("fc", (0, 3, 6, 9)), ("fc", (0, 3, 9, 12)),
             ("fc", (0, 3, 12, 16)), ("n", (0, 3)), ("fc", (1, 3, 0, 3)),
             ("t", (2, 0)), ("fc", (1, 3, 3, 6)), ("t", (2, 1)),
             ("fc", (1, 3, 6, 9)), ("t", (2, 2)), ("fc", (1, 3, 9, 12))],
        ]

        def emit_item(it):
            kind, arg = it
            if kind == "t":
                tail(arg[0], qts=[arg[1]])
            elif kind == "fc":
                flush_ctx(*arg)
            elif kind == "n":
                norm_ctx(arg[0], arg[1])
            elif kind == "v":
                v_proj(arg[0])
            else:
                qk_proj_half(wq_a, bq_s, qT, arg[0], arg[1], arg[2])

        for ci, (hp, qb4) in enumerate(combos):
            items = list(fillers[ci])
            for kc in range(KC):
                sc_exp(hp, qb4, kc)
                if kc >= 1 and items:
                    emit_item(items.pop(0))
            while items:
                emit_item(items.pop(0))
        tail(2, qts=[3])
        flush_ctx(1, 3, 12, KC)
        norm_ctx(1, 3, tails=True)

        work.release()
        psT.release()
        psC.release()
        psA.release()
        pers.release()

    nc.compile()
    return nc


def _get_nc():
    if "nc" not in _CACHE:
        _CACHE["nc"] = _build()
    return _CACHE["nc"]


def kernel(hidden_states, attention_mask, Wq, bq, Wk, bk, Wv, bv, Wo, bo):
    from concourse.bass_utils import run_bass_kernel_spmd

    hidden_states = np.asarray(hidden_states, np.float32)
    attention_mask = np.asarray(attention_mask, np.float32)
    Wq, Wk, Wv, Wo = (np.asarray(a, np.float32) for a in (Wq, Wk, Wv, Wo))
    bq, bk, bv, bo = (np.asarray(a, np.float32) for a in (bq, bk, bv, bo))

    nc = _get_nc()
    in_maps = []
    xTb = [np.ascontiguousarray(hidden_states[b].T).astype(np.float16)
           for b in range(B)]
    maskb = [np.ascontiguousarray(attention_mask[b, 0, 0, :])
             for b in range(B)]
    for c in range(NCORES):
        b, g = c // HPC, c % HPC
        cs = slice(g * COLS, (g + 1) * COLS)
        in_maps.append({
            "xT": xTb[b],
            "wq": np.ascontiguousarray(Wq[:, cs]).astype(np.float16),
            "wk": np.ascontiguousarray(Wk[:, cs]).astype(np.float16),
            "wv": np.ascontiguousarray(Wv[:, cs]).astype(np.float16),
            "wo": np.ascontiguousarray(Wo[cs, :]).astype(np.float16),
            "bq": np.ascontiguousarray(bq[cs]),
            "bk": np.ascontiguousarray(bk[cs]),
            "mask": maskb[b],
        })

    trace = bool(os.environ.get("KERNEL_TRACE"))
    kw = {}
    if trace:
        kw = dict(trace=True, tmpdir=os.environ.get("KERNEL_TRACE_DIR"))
    res = run_bass_kernel_spmd(nc, in_maps, list(range(NCORES)), **kw)
    _CACHE["last_result"] = res

    out = np.zeros((B, S, H), np.float32)
    for c in range(NCORES):
        out[c // HPC] += res.results[c]["out"]
    out += bv @ Wo + bo
    return out
